# revision 30
# baseline (speedup 1.0000x reference)
"""Bilinear-sampling + global average pooling on 8 Trainium2 NeuronCores.

Math: out[b,c] = mean_{h,w} bilinear(data[b,c], grid + ts*offset[b])
The gather indices/weights depend only on (b,h,w), never on c, so the whole
op is a linear functional over spatial positions applied per channel:

    out[b,c] = (1/S) * sum_s A[b,s] * data[b,c,s]      (s = flattened H*W)

where A is the scatter-accumulation of the four bilinear corner weights of
every sample point.  A is computed on host from `offset` (131K elements,
0.1% of `data`); the device kernel does the memory-bound weighted reduction
over the `data` tensor.  The 1/S mean and the final [B,C] assembly happen
on host.

The op is memory-bound, so `data` ships to HBM as fp16 (host cast): halves
DMA traffic vs fp32 at ~4e-4 relative error — far inside tolerance.

Device kernel (per core, 4 batches = 8 slabs of [128 channels, 4096]):
  - One 1 MiB HWDGE DMA per slab, deep-prefetched (all 8 issued up front on
    the Sync ring; A-row loads + result stores ride the Scalar ring).
  - A[b] rows are replicated across partitions by GPSIMD partition_broadcast
    operating on fp16 pairs packed as fp32 (halves the element count; DVE
    reads the tile back through a fp16 bitcast view).
  - Per slab, the weighted reduction runs as either
      DVE tensor_tensor multiply (2x perf mode) + ACT Copy-activation with
      accumulator (6 slabs), or
      DVE fused scalar_tensor_tensor with accum_out (2 slabs),
    chosen to balance DVE vs ACT engine time; each engine writes its own
    scratch/output tiles so no cross-engine false dependencies arise.
  - Two small DMAs store the per-slab fp32 column sums.
"""

import os
import sys

import numpy as np

for _p in ("/opt/trn_rl_repo", "/root/.axon_site/_ro/trn_rl_repo"):
    if os.path.isdir(_p) and _p not in sys.path:
        sys.path.append(_p)

import concourse.bacc as bacc
import concourse.mybir as mybir
import concourse.tile as tile
from concourse.bass_utils import run_bass_kernel_spmd

N_CORES = 8
B, C, H, W = 32, 256, 64, 64
S = H * W            # 4096 spatial positions
NB = B // N_CORES    # 4 batches per core
NCH = C // 128       # 2 channel halves of 128 partitions
NBC = NB * NCH       # 8 slabs of [128, S] per core
STT_SET = (1, 3, 5, 7)     # slabs reduced via fused DVE scalar_tensor_tensor
PE_BATCHES = (1, 2, 3)     # batches whose A-row broadcast runs on PE+ACT
MM_N = 512                 # moving-operand width per matmul (fp16 limit)
# PE-broadcast batches run first (their A tiles are ready earlier than the
# gpsimd-broadcast batch-0 tile); column order in the output tiles follows.
G_ORDER = tuple([g for g in range(NBC) if g // NCH in PE_BATCHES]
                + [g for g in range(NBC) if g // NCH not in PE_BATCHES])
TT_ORDER = tuple(g for g in G_ORDER if g not in STT_SET)
STT_ORDER = tuple(g for g in G_ORDER if g in STT_SET)
TT_SET = tuple(g for g in range(NBC) if g not in STT_SET)

_CACHE = {}
LAST_RESULTS = None  # BassKernelResults of the most recent run (for test.py)


def _build_nc():
    nc = bacc.Bacc("TRN2", target_bir_lowering=False, debug=False,
                   num_devices=N_CORES)
    f16 = mybir.dt.float16
    f32 = mybir.dt.float32
    x = nc.dram_tensor("x", [NBC, 128, S], f16, kind="ExternalInput")
    awp = nc.dram_tensor("awp", [NB, S // 2], f32, kind="ExternalInput")
    ones = nc.dram_tensor("ones", [1, 128], f16, kind="ExternalInput")
    ya = nc.dram_tensor("ya", [128, len(TT_SET)], f32, kind="ExternalOutput")
    yb = (nc.dram_tensor("yb", [128, len(STT_SET)], f32, kind="ExternalOutput")
          if STT_SET else None)
    xt, at, yat = x.ap(), awp.ap(), ya.ap()
    ybt = yb.ap() if yb is not None else None
    mult = mybir.AluOpType.mult

    with tile.TileContext(nc) as tc:
        with (
            tc.tile_pool(name="arow", bufs=NB) as arowp,
            tc.tile_pool(name="abc", bufs=1) as abcp,
            tc.tile_pool(name="data", bufs=NBC) as datap,
            tc.tile_pool(name="prod", bufs=3) as prodp,
            tc.tile_pool(name="junk", bufs=2) as junkp,
            tc.tile_pool(name="col", bufs=2) as colp,
            tc.tile_pool(name="ps", bufs=2, space="PSUM") as psp,
        ):
            junka = junkp.tile([128, S], f16)   # ACT scratch
            junkd = junkp.tile([128, S], f16)   # DVE-STT scratch
            colsa = colp.tile([128, len(TT_SET)], f32)
            colsd = (colp.tile([128, len(STT_SET)], f32, name="colsd")
                     if STT_SET else None)

            # A rows (fp16 packed as fp32 pairs): two on each HWDGE ring,
            # at the head of both queues, so all four land by ~8us and the
            # broadcast work can start as early as possible.
            a_sbs = []
            with tc.high_priority():
                for b in range(NB):
                    a_sb = arowp.tile([1, S // 2], f32)
                    eng = nc.sync if b < 2 else nc.scalar
                    eng.dma_start(a_sb[:], at[b : b + 1, :])
                    a_sbs.append(a_sb)
                onest = arowp.tile([1, 128], f16, name="onest", bufs=1)
                nc.gpsimd.memset(onest[:], 1.0)

            # Broadcast A rows across partitions.  Batch 0 uses GPSIMD
            # partition_broadcast on fp32-packed pairs (runs in the DVE-idle
            # warmup window; GpSimd and DVE serialize on a shared SBUF port,
            # so later batches instead ride PE (ones ⊗ row into PSUM) + ACT
            # copies, which touch neither DVE nor GpSimd ports.
            abqs = [None] * NB
            for b in range(NB):
                if b in PE_BATCHES:
                    abp = abcp.tile([128, S], f16, name=f"abp{b}", bufs=1)
                    rhs16 = a_sbs[b].bitcast(f16)       # [1, S] fp16 view
                    # the first-processed batch copies in 1024-col pieces so
                    # its A tile (the pipeline's start gate) is ready sooner
                    npc = 2 if b == PE_BATCHES[0] else 1
                    for k in range(S // 2048):
                        ps = psp.tile([128, 2048], f32, name="ps")
                        for h in range(2048 // MM_N):
                            c0 = k * 2048 + h * MM_N
                            nc.tensor.matmul(
                                ps[:, h * MM_N : (h + 1) * MM_N],
                                lhsT=onest[0:1, :],
                                rhs=rhs16[0:1, c0 : c0 + MM_N],
                                start=True, stop=True)
                        w = 2048 // npc
                        for p in range(npc):
                            nc.scalar.activation(
                                abp[:, k * 2048 + p * w :
                                    k * 2048 + (p + 1) * w],
                                ps[:, p * w : (p + 1) * w],
                                mybir.ActivationFunctionType.Copy)
                    abqs[b] = abp
                else:
                    abg = abcp.tile([128, S // 2], f32, name=f"abg{b}",
                                    bufs=1)
                    nc.gpsimd.partition_broadcast(abg[:], a_sbs[b][0:1, :])
                    abqs[b] = abg.bitcast(f16)          # [128, S] fp16 view

            # Process PE-broadcast batches first: ab1 is ready ~4us before
            # the gpsimd-broadcast ab0, so batch 0 goes last.  Data DMAs are
            # issued in the same order so the stream matches consumption.
            ds = {}
            for g in G_ORDER:
                d = datap.tile([128, S], f16, name=f"d{g}", bufs=1)
                nc.sync.dma_start(d[:], xt[g])
                ds[g] = d

            ai = di = 0
            for g in G_ORDER:
                b = g // NCH
                if g in STT_SET:
                    nc.vector.scalar_tensor_tensor(
                        out=junkd[:],
                        in0=ds[g][:],
                        scalar=1.0,
                        in1=abqs[b][:],
                        op0=mult,
                        op1=mult,
                        accum_out=colsd[:, di : di + 1],
                    )
                    di += 1
                else:
                    prod = prodp.tile([128, S], f16)
                    nc.vector.tensor_tensor(
                        out=prod[:], in0=ds[g][:], in1=abqs[b][:], op=mult)
                    nc.scalar.activation(
                        junka[:], prod[:],
                        mybir.ActivationFunctionType.Copy,
                        accum_out=colsa[:, ai : ai + 1])
                    ai += 1

            nc.scalar.dma_start(yat[:, :], colsa[:])
            if STT_SET:
                nc.sync.dma_start(ybt[:, :], colsd[:])

    nc.compile()
    return nc


def _weight_field(offset, trans_std):
    """A[b,s]: accumulated bilinear weights per source pixel (unscaled; the
    1/S mean is applied on host after download).  Mirrors the reference
    coordinate math in float32.
    """
    offset = np.asarray(offset, np.float32)
    ts = np.float32(min(max(float(trans_std), 0.001), 0.01))
    ii = np.arange(H, dtype=np.float32)[None, :, None]
    jj = np.arange(W, dtype=np.float32)[None, None, :]
    y = np.clip(ii + ts * offset[:, 0] * np.float32(H),
                np.float32(0.0), np.float32(H - 1))
    x = np.clip(jj + ts * offset[:, 1] * np.float32(W),
                np.float32(0.0), np.float32(W - 1))
    y0 = np.clip(np.floor(y).astype(np.int32), 0, H - 2)
    x0 = np.clip(np.floor(x).astype(np.int32), 0, W - 2)
    wy = (y - y0.astype(np.float32)).astype(np.float64)
    wx = (x - x0.astype(np.float32)).astype(np.float64)

    base = np.arange(offset.shape[0], dtype=np.int64)[:, None, None] * S
    i00 = (y0.astype(np.int64) * W + x0 + base).ravel()
    i01 = i00 + 1
    i10 = i00 + W
    i11 = i10 + 1
    n = offset.shape[0] * S
    acc = (
        np.bincount(i00, ((1 - wy) * (1 - wx)).ravel(), minlength=n)
        + np.bincount(i01, ((1 - wy) * wx).ravel(), minlength=n)
        + np.bincount(i10, (wy * (1 - wx)).ravel(), minlength=n)
        + np.bincount(i11, (wy * wx).ravel(), minlength=n)
    )
    return acc.astype(np.float32).reshape(offset.shape[0], S)


def _trace_available():
    """Trace only when the axon NTFF hook is registered (dev loop); in a
    bare harness environment this returns False and the run is untraced."""
    try:
        from antenv.axon_hooks import get_axon_ntff_profile_hook
    except ImportError:
        return False
    return get_axon_ntff_profile_hook() is not None


def kernel(data, offset, trans_std):
    global LAST_RESULTS
    data = np.asarray(data, np.float32)
    offset = np.asarray(offset, np.float32)
    ts = float(np.asarray(trans_std).reshape(()))

    aw16 = np.ascontiguousarray(
        _weight_field(offset, ts).astype(np.float16))    # [B, S] unscaled
    awp = aw16.view(np.float32)                          # [B, S//2] packed
    x16 = data.reshape(B, NCH, 128, S).astype(np.float16)

    if "nc" not in _CACHE:
        _CACHE["nc"] = _build_nc()
    nc = _CACHE["nc"]

    ones = np.ones((1, 128), np.float16)
    in_maps = []
    for i in range(N_CORES):
        xi = np.ascontiguousarray(
            x16[i * NB : (i + 1) * NB].reshape(NBC, 128, S))
        ai = np.ascontiguousarray(awp[i * NB : (i + 1) * NB])
        in_maps.append({"x": xi, "awp": ai, "ones": ones})

    res = run_bass_kernel_spmd(nc, in_maps, core_ids=list(range(N_CORES)),
                               trace=_trace_available())
    LAST_RESULTS = res
    # y[p, g] with g = b*NCH + ch → out[b, ch*128 + p], divided by S
    parts = []
    for i in range(N_CORES):
        ycols = np.empty((128, NBC), np.float32)
        ycols[:, list(TT_ORDER)] = res.results[i]["ya"]
        if STT_SET:
            ycols[:, list(STT_ORDER)] = res.results[i]["yb"]
        yi = ycols.reshape(128, NB, NCH)
        parts.append(np.transpose(yi, (1, 2, 0)).reshape(NB, C))
    out = np.concatenate(parts, axis=0) * np.float32(1.0 / S)
    return np.ascontiguousarray(out.astype(np.float32))


# revision 37
# speedup vs baseline: 1.2437x; 1.2437x over previous
"""Bilinear-sampling + global average pooling on 8 Trainium2 NeuronCores.

Math: out[b,c] = mean_{h,w} bilinear(data[b,c], grid + ts*offset[b])
The gather indices/weights depend only on (b,h,w), never on c, so the whole
op is a linear functional over spatial positions applied per channel:

    out[b,c] = (1/S) * sum_s A[b,s] * data[b,c,s]      (s = flattened H*W)

where A is the scatter-accumulation of the four bilinear corner weights of
every sample point.  A is computed on host from `offset` (131K elements,
0.1% of `data`); the device kernel does the memory-bound weighted reduction
over the `data` tensor.  The 1/S mean and the final [B,C] assembly happen
on host.

The op is memory-bound, so `data` ships to HBM as fp16 (host cast): halves
DMA traffic vs fp32 at ~4e-4 relative error — far inside tolerance.

Device kernel (per core, 4 batches = 8 slabs of [128 channels, 4096]):
  - One 1 MiB HWDGE DMA per slab, deep-prefetched (all 8 issued up front on
    the Sync ring; A-row loads + result stores ride the Scalar ring).
  - A[b] rows are replicated across partitions by GPSIMD partition_broadcast
    operating on fp16 pairs packed as fp32 (halves the element count; DVE
    reads the tile back through a fp16 bitcast view).
  - Per slab, the weighted reduction runs as either
      DVE tensor_tensor multiply (2x perf mode) + ACT Copy-activation with
      accumulator (6 slabs), or
      DVE fused scalar_tensor_tensor with accum_out (2 slabs),
    chosen to balance DVE vs ACT engine time; each engine writes its own
    scratch/output tiles so no cross-engine false dependencies arise.
  - Two small DMAs store the per-slab fp32 column sums.
"""

import os
import sys

import numpy as np

for _p in ("/opt/trn_rl_repo", "/root/.axon_site/_ro/trn_rl_repo"):
    if os.path.isdir(_p) and _p not in sys.path:
        sys.path.append(_p)

import concourse.bacc as bacc
import concourse.mybir as mybir
import concourse.tile as tile
from concourse.bass_utils import run_bass_kernel_spmd

N_CORES = 8
B, C, H, W = 32, 256, 64, 64
S = H * W            # 4096 spatial positions
NB = B // N_CORES    # 4 batches per core
NCH = C // 128       # 2 channel halves of 128 partitions
NBC = NB * NCH       # 8 slabs of [128, S] per core
MM_N = 512                 # moving-operand width per matmul (fp16 limit)
# All four A-row broadcasts ride PE (ones ⊗ row into PSUM).  Batches 1-3 are
# copied to SBUF fp16 (ACT) and processed first; batch 0's two slabs instead
# run fused STT reading the PSUM broadcast directly (no copies) and go last.
# Slab roles: TT+ACT reduce for TT_ORDER, fused DVE STT for STT_ORDER
# (SBUF A tile), and per-half PSUM-direct STT for slabs 0 and 1.
G_ORDER = (2, 3, 4, 5, 6, 7, 0, 1)
TT_ORDER = (2, 4, 5, 6)    # → ya columns, in this order
STT_ORDER = (3, 7)         # → yb columns 0-1; yb cols 2-5 are the four
                           #   half-sums of slabs 0 and 1 (host adds pairs)
STT_SET = (3, 7)
PSUM_SET = (0, 1)
NYB = len(STT_ORDER) + 2 * len(PSUM_SET)
TT_SET = tuple(g for g in range(NBC) if g not in STT_SET)

_CACHE = {}
LAST_RESULTS = None  # BassKernelResults of the most recent run (for test.py)


def _build_nc():
    nc = bacc.Bacc("TRN2", target_bir_lowering=False, debug=False,
                   num_devices=N_CORES)
    f16 = mybir.dt.float16
    f32 = mybir.dt.float32
    x = nc.dram_tensor("x", [NBC, 128, S], f16, kind="ExternalInput")
    # each row: 64 fp32 = 128 fp16 ones, then the fp16 A row packed as fp32
    awp = nc.dram_tensor("awp", [NB, 64 + S // 2], f32, kind="ExternalInput")
    ya = nc.dram_tensor("ya", [128, len(TT_ORDER)], f32, kind="ExternalOutput")
    yb = nc.dram_tensor("yb", [128, NYB], f32, kind="ExternalOutput")
    xt, at, yat, ybt = x.ap(), awp.ap(), ya.ap(), yb.ap()
    mult = mybir.AluOpType.mult

    with tile.TileContext(nc) as tc:
        with (
            tc.tile_pool(name="arow", bufs=NB) as arowp,
            tc.tile_pool(name="abc", bufs=1) as abcp,
            tc.tile_pool(name="data", bufs=NBC) as datap,
            tc.tile_pool(name="prod", bufs=3) as prodp,
            tc.tile_pool(name="junk", bufs=2) as junkp,
            tc.tile_pool(name="col", bufs=2) as colp,
            tc.tile_pool(name="ps", bufs=2, space="PSUM") as psp,
        ):
            junka = junkp.tile([128, S], f16)   # ACT scratch
            junkd = junkp.tile([128, S], f16)   # DVE-STT scratch
            colsa = colp.tile([128, len(TT_ORDER)], f32)
            colsd = colp.tile([128, NYB], f32, name="colsd")

            # A-row loads (ones prefix + fp16 pairs packed as fp32): batch 1
            # first on the Sync ring — it gates the whole pipeline — then
            # batches 2,3 on Scalar and batch 0 last.
            a_sbs = [None] * NB
            with tc.high_priority():
                for b, eng in ((1, nc.sync), (2, nc.scalar),
                               (3, nc.scalar), (0, nc.sync)):
                    a_sb = arowp.tile([1, 64 + S // 2], f32,
                                      name=f"asb{b}", bufs=1)
                    eng.dma_start(a_sb[:], at[b : b + 1, :])
                    a_sbs[b] = a_sb
            onest = a_sbs[1][:, 0:64].bitcast(f16)      # [1, 128] ones

            # Broadcast A rows across partitions on PE (ones ⊗ row → PSUM).
            # Batches 1-3 are copied to SBUF fp16 by ACT; batch 0's PSUM
            # tiles are consumed directly by its fused STT slabs (GpSimd
            # stays idle — it would lock DVE out of a shared SBUF port).
            abqs = [None] * NB
            ps0 = []
            for b in (1, 2, 3, 0):
                rhs16 = a_sbs[b][:, 64:].bitcast(f16)   # [1, S] fp16 view
                if b == 0:
                    for k in range(S // 2048):
                        ps = psp.tile([128, 2048], f32, name="ps")
                        for h in range(2048 // MM_N):
                            c0 = k * 2048 + h * MM_N
                            nc.tensor.matmul(
                                ps[:, h * MM_N : (h + 1) * MM_N],
                                lhsT=onest[0:1, :],
                                rhs=rhs16[0:1, c0 : c0 + MM_N],
                                start=True, stop=True)
                        ps0.append(ps)
                    continue
                abp = abcp.tile([128, S], f16, name=f"abp{b}", bufs=1)
                for k in range(S // 2048):
                    ps = psp.tile([128, 2048], f32, name="ps")
                    for h in range(2048 // MM_N):
                        c0 = k * 2048 + h * MM_N
                        nc.tensor.matmul(
                            ps[:, h * MM_N : (h + 1) * MM_N],
                            lhsT=onest[0:1, :],
                            rhs=rhs16[0:1, c0 : c0 + MM_N],
                            start=True, stop=True)
                    nc.scalar.activation(
                        abp[:, k * 2048 : (k + 1) * 2048], ps[:],
                        mybir.ActivationFunctionType.Copy)
                abqs[b] = abp

            # Process PE-broadcast batches first: ab1 is ready ~4us before
            # the gpsimd-broadcast ab0, so batch 0 goes last.  Data DMAs are
            # issued in the same order so the stream matches consumption.
            ds = {}
            for g in G_ORDER:
                d = datap.tile([128, S], f16, name=f"d{g}", bufs=1)
                nc.sync.dma_start(d[:], xt[g])
                ds[g] = d

            ai = di = 0
            for g in G_ORDER:
                b = g // NCH
                if g in PSUM_SET:
                    # batch-0 slab: fused STT per 2048-half against the PSUM
                    # broadcast tiles; the two half-sums are added on host.
                    for h in range(2):
                        nc.vector.scalar_tensor_tensor(
                            out=junkd[:, 0:2048],
                            in0=ds[g][:, h * 2048 : (h + 1) * 2048],
                            scalar=1.0,
                            in1=ps0[h][:],
                            op0=mult,
                            op1=mult,
                            accum_out=colsd[:, di : di + 1],
                        )
                        di += 1
                elif g in STT_SET:
                    nc.vector.scalar_tensor_tensor(
                        out=junkd[:],
                        in0=ds[g][:],
                        scalar=1.0,
                        in1=abqs[b][:],
                        op0=mult,
                        op1=mult,
                        accum_out=colsd[:, di : di + 1],
                    )
                    di += 1
                else:
                    prod = prodp.tile([128, S], f16)
                    nc.vector.tensor_tensor(
                        out=prod[:], in0=ds[g][:], in1=abqs[b][:], op=mult)
                    nc.scalar.activation(
                        junka[:], prod[:],
                        mybir.ActivationFunctionType.Copy,
                        accum_out=colsa[:, ai : ai + 1])
                    ai += 1

            nc.scalar.dma_start(yat[:, :], colsa[:])
            nc.sync.dma_start(ybt[:, :], colsd[:])

    nc.compile()
    return nc


def _weight_field(offset, trans_std):
    """A[b,s]: accumulated bilinear weights per source pixel (unscaled; the
    1/S mean is applied on host after download).  Mirrors the reference
    coordinate math in float32.
    """
    offset = np.asarray(offset, np.float32)
    ts = np.float32(min(max(float(trans_std), 0.001), 0.01))
    ii = np.arange(H, dtype=np.float32)[None, :, None]
    jj = np.arange(W, dtype=np.float32)[None, None, :]
    y = np.clip(ii + ts * offset[:, 0] * np.float32(H),
                np.float32(0.0), np.float32(H - 1))
    x = np.clip(jj + ts * offset[:, 1] * np.float32(W),
                np.float32(0.0), np.float32(W - 1))
    y0 = np.clip(np.floor(y).astype(np.int32), 0, H - 2)
    x0 = np.clip(np.floor(x).astype(np.int32), 0, W - 2)
    wy = (y - y0.astype(np.float32)).astype(np.float64)
    wx = (x - x0.astype(np.float32)).astype(np.float64)

    base = np.arange(offset.shape[0], dtype=np.int64)[:, None, None] * S
    i00 = (y0.astype(np.int64) * W + x0 + base).ravel()
    i01 = i00 + 1
    i10 = i00 + W
    i11 = i10 + 1
    n = offset.shape[0] * S
    acc = (
        np.bincount(i00, ((1 - wy) * (1 - wx)).ravel(), minlength=n)
        + np.bincount(i01, ((1 - wy) * wx).ravel(), minlength=n)
        + np.bincount(i10, (wy * (1 - wx)).ravel(), minlength=n)
        + np.bincount(i11, (wy * wx).ravel(), minlength=n)
    )
    return acc.astype(np.float32).reshape(offset.shape[0], S)


def _trace_available():
    """Trace only when the axon NTFF hook is registered (dev loop); in a
    bare harness environment this returns False and the run is untraced."""
    try:
        from antenv.axon_hooks import get_axon_ntff_profile_hook
    except ImportError:
        return False
    return get_axon_ntff_profile_hook() is not None


def _assemble_core(ya, yb):
    """[128, NBC] column sums from the two output tiles of one core."""
    ycols = np.empty((128, NBC), np.float32)
    ycols[:, list(TT_ORDER)] = ya
    ycols[:, STT_ORDER[0]] = yb[:, 0]
    ycols[:, STT_ORDER[1]] = yb[:, 1]
    ycols[:, PSUM_SET[0]] = yb[:, 2] + yb[:, 3]
    ycols[:, PSUM_SET[1]] = yb[:, 4] + yb[:, 5]
    return ycols


def kernel(data, offset, trans_std):
    global LAST_RESULTS
    data = np.asarray(data, np.float32)
    offset = np.asarray(offset, np.float32)
    ts = float(np.asarray(trans_std).reshape(()))

    aw16 = np.ascontiguousarray(
        _weight_field(offset, ts).astype(np.float16))    # [B, S] unscaled
    ones_pack = np.ones(128, np.float16).view(np.float32)      # [64]
    awp = np.concatenate(
        [np.tile(ones_pack, (B, 1)), aw16.view(np.float32)], axis=1)
    x16 = data.reshape(B, NCH, 128, S).astype(np.float16)

    if "nc" not in _CACHE:
        _CACHE["nc"] = _build_nc()
    nc = _CACHE["nc"]

    in_maps = []
    for i in range(N_CORES):
        xi = np.ascontiguousarray(
            x16[i * NB : (i + 1) * NB].reshape(NBC, 128, S))
        ai = np.ascontiguousarray(awp[i * NB : (i + 1) * NB])
        in_maps.append({"x": xi, "awp": ai})

    res = run_bass_kernel_spmd(nc, in_maps, core_ids=list(range(N_CORES)),
                               trace=_trace_available())
    LAST_RESULTS = res
    # y[p, g] with g = b*NCH + ch → out[b, ch*128 + p], divided by S
    parts = []
    for i in range(N_CORES):
        ycols = _assemble_core(res.results[i]["ya"], res.results[i]["yb"])
        yi = ycols.reshape(128, NB, NCH)
        parts.append(np.transpose(yi, (1, 2, 0)).reshape(NB, C))
    out = np.concatenate(parts, axis=0) * np.float32(1.0 / S)
    return np.ascontiguousarray(out.astype(np.float32))


# revision 39
# speedup vs baseline: 1.2466x; 1.0023x over previous
"""Bilinear-sampling + global average pooling on 8 Trainium2 NeuronCores.

Math: out[b,c] = mean_{h,w} bilinear(data[b,c], grid + ts*offset[b])
The gather indices/weights depend only on (b,h,w), never on c, so the whole
op is a linear functional over spatial positions applied per channel:

    out[b,c] = (1/S) * sum_s A[b,s] * data[b,c,s]      (s = flattened H*W)

where A is the scatter-accumulation of the four bilinear corner weights of
every sample point.  A is computed on host from `offset` (131K elements,
0.1% of `data`); the device kernel does the memory-bound weighted reduction
over the `data` tensor.  The 1/S mean and the final [B,C] assembly happen
on host.

The op is memory-bound, so `data` ships to HBM as fp16 (host cast): halves
DMA traffic vs fp32 at ~4e-4 relative error — far inside tolerance.

Device kernel (per core, 4 batches = 8 slabs of [128 channels, 4096]):
  - One 1 MiB HWDGE DMA per slab, deep-prefetched (all 8 issued up front on
    the Sync ring; A-row loads + result stores split across both rings).
  - A[b] rows are replicated across partitions by the TENSOR engine
    (ones[1,128] ⊗ row, 512-wide fp16 matmuls into PSUM).  Each A-row DMA
    carries a 128-one fp16 prefix so the stationary operand needs no
    separate load.  Batches 1-3 are cast-copied PSUM→SBUF fp16 by ACT;
    batch 0's PSUM tiles are consumed in place.  GPSIMD stays idle: it
    shares an exclusively-locked SBUF port with DVE, so any gpsimd op
    stretches concurrent DVE work ~2.3x.
  - Per slab, the weighted reduction runs as one of (balancing DVE vs ACT):
      DVE tensor_tensor multiply (2x perf mode) + ACT Copy-activation with
      accumulator (4 slabs),
      DVE fused scalar_tensor_tensor with accum_out (2 slabs), or
      per-2048-half fused STT reading the PSUM broadcast directly (batch 0;
      the half-sums are added on host).
  - Two small DMAs (one per ring) store the fp32 column sums.
"""

import os
import sys

import numpy as np

for _p in ("/opt/trn_rl_repo", "/root/.axon_site/_ro/trn_rl_repo"):
    if os.path.isdir(_p) and _p not in sys.path:
        sys.path.append(_p)

import concourse.bacc as bacc
import concourse.mybir as mybir
import concourse.tile as tile
from concourse.bass_utils import run_bass_kernel_spmd

N_CORES = 8
B, C, H, W = 32, 256, 64, 64
S = H * W            # 4096 spatial positions
NB = B // N_CORES    # 4 batches per core
NCH = C // 128       # 2 channel halves of 128 partitions
NBC = NB * NCH       # 8 slabs of [128, S] per core
MM_N = 512                 # moving-operand width per matmul (fp16 limit)
# All four A-row broadcasts ride PE (ones ⊗ row into PSUM).  Batches 1-3 are
# copied to SBUF fp16 (ACT) and processed first; batch 0's two slabs instead
# run fused STT reading the PSUM broadcast directly (no copies) and go last.
# Slab roles: TT+ACT reduce for TT_ORDER, fused DVE STT for STT_ORDER
# (SBUF A tile), and per-half PSUM-direct STT for slabs 0 and 1.
G_ORDER = (2, 3, 4, 5, 6, 7, 0, 1)
TT_ORDER = (2, 4, 5, 6)    # → ya columns, in this order
STT_ORDER = (3, 7)         # → yb columns 0-1; yb cols 2-5 are the four
                           #   half-sums of slabs 0 and 1 (host adds pairs)
STT_SET = (3, 7)
PSUM_SET = (0, 1)
NYB = len(STT_ORDER) + 2 * len(PSUM_SET)

_CACHE = {}
LAST_RESULTS = None  # BassKernelResults of the most recent run (for test.py)


def _build_nc():
    nc = bacc.Bacc("TRN2", target_bir_lowering=False, debug=False,
                   num_devices=N_CORES)
    f16 = mybir.dt.float16
    f32 = mybir.dt.float32
    x = nc.dram_tensor("x", [NBC, 128, S], f16, kind="ExternalInput")
    # each row: 64 fp32 = 128 fp16 ones, then the fp16 A row packed as fp32
    awp = nc.dram_tensor("awp", [NB, 64 + S // 2], f32, kind="ExternalInput")
    ya = nc.dram_tensor("ya", [128, len(TT_ORDER)], f32, kind="ExternalOutput")
    yb = nc.dram_tensor("yb", [128, NYB], f32, kind="ExternalOutput")
    xt, at, yat, ybt = x.ap(), awp.ap(), ya.ap(), yb.ap()
    mult = mybir.AluOpType.mult

    with tile.TileContext(nc) as tc:
        with (
            tc.tile_pool(name="arow", bufs=NB) as arowp,
            tc.tile_pool(name="abc", bufs=1) as abcp,
            tc.tile_pool(name="data", bufs=NBC) as datap,
            tc.tile_pool(name="prod", bufs=3) as prodp,
            tc.tile_pool(name="junk", bufs=2) as junkp,
            tc.tile_pool(name="col", bufs=2) as colp,
            tc.tile_pool(name="ps", bufs=2, space="PSUM") as psp,
        ):
            junka = junkp.tile([128, S], f16)   # ACT scratch
            junkd = junkp.tile([128, S], f16)   # DVE-STT scratch
            colsa = colp.tile([128, len(TT_ORDER)], f32)
            colsd = colp.tile([128, NYB], f32, name="colsd")

            # A-row loads (ones prefix + fp16 pairs packed as fp32): batch 1
            # first on the Sync ring — it gates the whole pipeline — then
            # batches 2,3 on Scalar and batch 0 last.
            a_sbs = [None] * NB
            with tc.high_priority():
                for b, eng in ((1, nc.sync), (2, nc.scalar),
                               (3, nc.scalar), (0, nc.sync)):
                    a_sb = arowp.tile([1, 64 + S // 2], f32,
                                      name=f"asb{b}", bufs=1)
                    eng.dma_start(a_sb[:], at[b : b + 1, :])
                    a_sbs[b] = a_sb
            onest = a_sbs[1][:, 0:64].bitcast(f16)      # [1, 128] ones

            # Broadcast A rows across partitions on PE (ones ⊗ row → PSUM).
            # Batches 1-3 are copied to SBUF fp16 by ACT; batch 0's PSUM
            # tiles are consumed directly by its fused STT slabs (GpSimd
            # stays idle — it would lock DVE out of a shared SBUF port).
            abqs = [None] * NB
            ps0 = []
            for b in (1, 2, 3, 0):
                rhs16 = a_sbs[b][:, 64:].bitcast(f16)   # [1, S] fp16 view
                if b == 0:
                    for k in range(S // 2048):
                        ps = psp.tile([128, 2048], f32, name="ps")
                        for h in range(2048 // MM_N):
                            c0 = k * 2048 + h * MM_N
                            nc.tensor.matmul(
                                ps[:, h * MM_N : (h + 1) * MM_N],
                                lhsT=onest[0:1, :],
                                rhs=rhs16[0:1, c0 : c0 + MM_N],
                                start=True, stop=True)
                        ps0.append(ps)
                    continue
                abp = abcp.tile([128, S], f16, name=f"abp{b}", bufs=1)
                for k in range(S // 2048):
                    ps = psp.tile([128, 2048], f32, name="ps")
                    for h in range(2048 // MM_N):
                        c0 = k * 2048 + h * MM_N
                        nc.tensor.matmul(
                            ps[:, h * MM_N : (h + 1) * MM_N],
                            lhsT=onest[0:1, :],
                            rhs=rhs16[0:1, c0 : c0 + MM_N],
                            start=True, stop=True)
                    nc.scalar.activation(
                        abp[:, k * 2048 : (k + 1) * 2048], ps[:],
                        mybir.ActivationFunctionType.Copy)
                abqs[b] = abp

            # Process PE-broadcast batches first: ab1 is ready ~4us before
            # the gpsimd-broadcast ab0, so batch 0 goes last.  Data DMAs are
            # issued in the same order so the stream matches consumption.
            ds = {}
            for g in G_ORDER:
                d = datap.tile([128, S], f16, name=f"d{g}", bufs=1)
                nc.sync.dma_start(d[:], xt[g])
                ds[g] = d

            ai = di = 0
            for g in G_ORDER:
                b = g // NCH
                if g in PSUM_SET:
                    # batch-0 slab: fused STT per 2048-half against the PSUM
                    # broadcast tiles; the two half-sums are added on host.
                    for h in range(2):
                        nc.vector.scalar_tensor_tensor(
                            out=junkd[:, 0:2048],
                            in0=ds[g][:, h * 2048 : (h + 1) * 2048],
                            scalar=1.0,
                            in1=ps0[h][:],
                            op0=mult,
                            op1=mult,
                            accum_out=colsd[:, di : di + 1],
                        )
                        di += 1
                elif g in STT_SET:
                    nc.vector.scalar_tensor_tensor(
                        out=junkd[:],
                        in0=ds[g][:],
                        scalar=1.0,
                        in1=abqs[b][:],
                        op0=mult,
                        op1=mult,
                        accum_out=colsd[:, di : di + 1],
                    )
                    di += 1
                else:
                    prod = prodp.tile([128, S], f16)
                    nc.vector.tensor_tensor(
                        out=prod[:], in0=ds[g][:], in1=abqs[b][:], op=mult)
                    nc.scalar.activation(
                        junka[:], prod[:],
                        mybir.ActivationFunctionType.Copy,
                        accum_out=colsa[:, ai : ai + 1])
                    ai += 1

            nc.scalar.dma_start(yat[:, :], colsa[:])
            nc.sync.dma_start(ybt[:, :], colsd[:])

    nc.compile()
    return nc


def _weight_field(offset, trans_std):
    """A[b,s]: accumulated bilinear weights per source pixel (unscaled; the
    1/S mean is applied on host after download).  Mirrors the reference
    coordinate math in float32.
    """
    offset = np.asarray(offset, np.float32)
    ts = np.float32(min(max(float(trans_std), 0.001), 0.01))
    ii = np.arange(H, dtype=np.float32)[None, :, None]
    jj = np.arange(W, dtype=np.float32)[None, None, :]
    y = np.clip(ii + ts * offset[:, 0] * np.float32(H),
                np.float32(0.0), np.float32(H - 1))
    x = np.clip(jj + ts * offset[:, 1] * np.float32(W),
                np.float32(0.0), np.float32(W - 1))
    y0 = np.clip(np.floor(y).astype(np.int32), 0, H - 2)
    x0 = np.clip(np.floor(x).astype(np.int32), 0, W - 2)
    wy = (y - y0.astype(np.float32)).astype(np.float64)
    wx = (x - x0.astype(np.float32)).astype(np.float64)

    base = np.arange(offset.shape[0], dtype=np.int64)[:, None, None] * S
    i00 = (y0.astype(np.int64) * W + x0 + base).ravel()
    i01 = i00 + 1
    i10 = i00 + W
    i11 = i10 + 1
    n = offset.shape[0] * S
    acc = (
        np.bincount(i00, ((1 - wy) * (1 - wx)).ravel(), minlength=n)
        + np.bincount(i01, ((1 - wy) * wx).ravel(), minlength=n)
        + np.bincount(i10, (wy * (1 - wx)).ravel(), minlength=n)
        + np.bincount(i11, (wy * wx).ravel(), minlength=n)
    )
    return acc.astype(np.float32).reshape(offset.shape[0], S)


def _trace_available():
    """Trace only when the axon NTFF hook is registered (dev loop); in a
    bare harness environment this returns False and the run is untraced."""
    try:
        from antenv.axon_hooks import get_axon_ntff_profile_hook
    except ImportError:
        return False
    return get_axon_ntff_profile_hook() is not None


def _assemble_core(ya, yb):
    """[128, NBC] column sums from the two output tiles of one core."""
    ycols = np.empty((128, NBC), np.float32)
    ycols[:, list(TT_ORDER)] = ya
    ycols[:, STT_ORDER[0]] = yb[:, 0]
    ycols[:, STT_ORDER[1]] = yb[:, 1]
    ycols[:, PSUM_SET[0]] = yb[:, 2] + yb[:, 3]
    ycols[:, PSUM_SET[1]] = yb[:, 4] + yb[:, 5]
    return ycols


def kernel(data, offset, trans_std):
    global LAST_RESULTS
    data = np.asarray(data, np.float32)
    offset = np.asarray(offset, np.float32)
    ts = float(np.asarray(trans_std).reshape(()))

    aw16 = np.ascontiguousarray(
        _weight_field(offset, ts).astype(np.float16))    # [B, S] unscaled
    ones_pack = np.ones(128, np.float16).view(np.float32)      # [64]
    awp = np.concatenate(
        [np.tile(ones_pack, (B, 1)), aw16.view(np.float32)], axis=1)
    x16 = data.reshape(B, NCH, 128, S).astype(np.float16)

    if "nc" not in _CACHE:
        _CACHE["nc"] = _build_nc()
    nc = _CACHE["nc"]

    in_maps = []
    for i in range(N_CORES):
        xi = np.ascontiguousarray(
            x16[i * NB : (i + 1) * NB].reshape(NBC, 128, S))
        ai = np.ascontiguousarray(awp[i * NB : (i + 1) * NB])
        in_maps.append({"x": xi, "awp": ai})

    res = run_bass_kernel_spmd(nc, in_maps, core_ids=list(range(N_CORES)),
                               trace=_trace_available())
    LAST_RESULTS = res
    # y[p, g] with g = b*NCH + ch → out[b, ch*128 + p], divided by S
    parts = []
    for i in range(N_CORES):
        ycols = _assemble_core(res.results[i]["ya"], res.results[i]["yb"])
        yi = ycols.reshape(128, NB, NCH)
        parts.append(np.transpose(yi, (1, 2, 0)).reshape(NB, C))
    out = np.concatenate(parts, axis=0) * np.float32(1.0 / S)
    return np.ascontiguousarray(out.astype(np.float32))


# revision 40
# speedup vs baseline: 1.2481x; 1.0012x over previous
"""Bilinear-sampling + global average pooling on 8 Trainium2 NeuronCores.

Math: out[b,c] = mean_{h,w} bilinear(data[b,c], grid + ts*offset[b])
The gather indices/weights depend only on (b,h,w), never on c, so the whole
op is a linear functional over spatial positions applied per channel:

    out[b,c] = (1/S) * sum_s A[b,s] * data[b,c,s]      (s = flattened H*W)

where A is the scatter-accumulation of the four bilinear corner weights of
every sample point.  A is computed on host from `offset` (131K elements,
0.1% of `data`); the device kernel does the memory-bound weighted reduction
over the `data` tensor.  The 1/S mean and the final [B,C] assembly happen
on host.

The op is memory-bound, so `data` ships to HBM as fp16 (host cast): halves
DMA traffic vs fp32 at ~4e-4 relative error — far inside tolerance.

Device kernel (per core, 4 batches = 8 slabs of [128 channels, 4096]):
  - One 1 MiB HWDGE DMA per slab, deep-prefetched (all 8 issued up front on
    the Sync ring; A-row loads + result stores split across both rings).
  - A[b] rows are replicated across partitions by the TENSOR engine
    (ones[1,128] ⊗ row, 512-wide fp16 matmuls into PSUM).  Each A-row DMA
    carries a 128-one fp16 prefix so the stationary operand needs no
    separate load.  Batches 1-3 are cast-copied PSUM→SBUF fp16 by ACT;
    batch 0's PSUM tiles are consumed in place.  GPSIMD stays idle: it
    shares an exclusively-locked SBUF port with DVE, so any gpsimd op
    stretches concurrent DVE work ~2.3x.
  - Per slab, the weighted reduction runs as one of (balancing DVE vs ACT):
      DVE tensor_tensor multiply (2x perf mode) + ACT Copy-activation with
      accumulator (4 slabs),
      DVE fused scalar_tensor_tensor with accum_out (2 slabs), or
      per-2048-half fused STT reading the PSUM broadcast directly (batch 0;
      the half-sums are added on host).
  - Two small DMAs (one per ring) store the fp32 column sums.
"""

import os
import sys

import numpy as np

for _p in ("/opt/trn_rl_repo", "/root/.axon_site/_ro/trn_rl_repo"):
    if os.path.isdir(_p) and _p not in sys.path:
        sys.path.append(_p)

import concourse.bacc as bacc
import concourse.mybir as mybir
import concourse.tile as tile
from concourse.bass_utils import run_bass_kernel_spmd

N_CORES = 8
B, C, H, W = 32, 256, 64, 64
S = H * W            # 4096 spatial positions
NB = B // N_CORES    # 4 batches per core
NCH = C // 128       # 2 channel halves of 128 partitions
NBC = NB * NCH       # 8 slabs of [128, S] per core
MM_N = 512                 # moving-operand width per matmul (fp16 limit)
# All four A-row broadcasts ride PE (ones ⊗ row into PSUM).  Batches 1-3 are
# copied to SBUF fp16 (ACT) and processed first; batch 0's two slabs instead
# run fused STT reading the PSUM broadcast directly (no copies) and go last.
# Slab roles: TT+ACT reduce for TT_ORDER, fused DVE STT for STT_ORDER
# (SBUF A tile), and per-half PSUM-direct STT for slabs 0 and 1.
G_ORDER = (2, 3, 4, 5, 6, 7, 0, 1)
TT_ORDER = (2, 4, 5, 6)    # → ya columns, in this order
STT_ORDER = (3, 7)         # → yb columns 0-1; yb cols 2-5 are the four
                           #   half-sums of slabs 0 and 1 (host adds pairs)
STT_SET = (3, 7)
PSUM_SET = (0, 1)
NYB = len(STT_ORDER) + 2 * len(PSUM_SET)

_CACHE = {}
LAST_RESULTS = None  # BassKernelResults of the most recent run (for test.py)


def _build_nc():
    nc = bacc.Bacc("TRN2", target_bir_lowering=False, debug=False,
                   num_devices=N_CORES)
    f16 = mybir.dt.float16
    f32 = mybir.dt.float32
    x = nc.dram_tensor("x", [NBC, 128, S], f16, kind="ExternalInput")
    # each row: 64 fp32 = 128 fp16 ones, then the fp16 A row packed as fp32
    awp = nc.dram_tensor("awp", [NB, 64 + S // 2], f32, kind="ExternalInput")
    ya = nc.dram_tensor("ya", [128, len(TT_ORDER)], f32, kind="ExternalOutput")
    yb = nc.dram_tensor("yb", [128, NYB], f32, kind="ExternalOutput")
    xt, at, yat, ybt = x.ap(), awp.ap(), ya.ap(), yb.ap()
    mult = mybir.AluOpType.mult

    with tile.TileContext(nc) as tc:
        with (
            tc.tile_pool(name="arow", bufs=NB) as arowp,
            tc.tile_pool(name="abc", bufs=1) as abcp,
            tc.tile_pool(name="data", bufs=NBC) as datap,
            tc.tile_pool(name="prod", bufs=3) as prodp,
            tc.tile_pool(name="junk", bufs=2) as junkp,
            tc.tile_pool(name="col", bufs=2) as colp,
            tc.tile_pool(name="ps", bufs=2, space="PSUM") as psp,
        ):
            junka = junkp.tile([128, S], f16)   # ACT scratch
            junkd = junkp.tile([128, S], f16)   # DVE-STT scratch
            colsa = colp.tile([128, len(TT_ORDER)], f32)
            colsd = colp.tile([128, NYB], f32, name="colsd")

            # A-row loads (ones prefix + fp16 pairs packed as fp32): batch 1
            # first on the Sync ring — it gates the whole pipeline — then
            # batches 2,3 on Scalar and batch 0 last.
            a_sbs = [None] * NB
            with tc.high_priority():
                for b, eng in ((1, nc.sync), (2, nc.scalar),
                               (3, nc.scalar), (0, nc.sync)):
                    a_sb = arowp.tile([1, 64 + S // 2], f32,
                                      name=f"asb{b}", bufs=1)
                    eng.dma_start(a_sb[:], at[b : b + 1, :])
                    a_sbs[b] = a_sb
            onest = a_sbs[1][:, 0:64].bitcast(f16)      # [1, 128] ones

            # Broadcast A rows across partitions on PE (ones ⊗ row → PSUM).
            # Batches 1-3 are copied to SBUF fp16 by ACT; batch 0's PSUM
            # tiles are consumed directly by its fused STT slabs (GpSimd
            # stays idle — it would lock DVE out of a shared SBUF port).
            abqs = [None] * NB
            ps0 = []
            for b in (1, 2, 3, 0):
                rhs16 = a_sbs[b][:, 64:].bitcast(f16)   # [1, S] fp16 view
                if b == 0:
                    for k in range(S // 2048):
                        ps = psp.tile([128, 2048], f32, name="ps")
                        for h in range(2048 // MM_N):
                            c0 = k * 2048 + h * MM_N
                            nc.tensor.matmul(
                                ps[:, h * MM_N : (h + 1) * MM_N],
                                lhsT=onest[0:1, :],
                                rhs=rhs16[0:1, c0 : c0 + MM_N],
                                start=True, stop=True)
                        ps0.append(ps)
                    continue
                abp = abcp.tile([128, S], f16, name=f"abp{b}", bufs=1)
                for k in range(S // 2048):
                    ps = psp.tile([128, 2048], f32, name="ps")
                    for h in range(2048 // MM_N):
                        c0 = k * 2048 + h * MM_N
                        nc.tensor.matmul(
                            ps[:, h * MM_N : (h + 1) * MM_N],
                            lhsT=onest[0:1, :],
                            rhs=rhs16[0:1, c0 : c0 + MM_N],
                            start=True, stop=True)
                    nc.scalar.activation(
                        abp[:, k * 2048 : (k + 1) * 2048], ps[:],
                        mybir.ActivationFunctionType.Copy)
                abqs[b] = abp

            # Process PE-broadcast batches first: ab1 is ready ~4us before
            # the gpsimd-broadcast ab0, so batch 0 goes last.  Data DMAs are
            # issued in the same order so the stream matches consumption.
            ds = {}
            for g in G_ORDER:
                d = datap.tile([128, S], f16, name=f"d{g}", bufs=1)
                nc.sync.dma_start(d[:], xt[g])
                ds[g] = d

            ai = di = 0
            for g in G_ORDER:
                b = g // NCH
                if g in PSUM_SET:
                    # batch-0 slab: fused STT per 2048-half against the PSUM
                    # broadcast tiles; the two half-sums are added on host.
                    for h in range(2):
                        nc.vector.scalar_tensor_tensor(
                            out=junkd[:, 0:2048],
                            in0=ds[g][:, h * 2048 : (h + 1) * 2048],
                            scalar=1.0,
                            in1=ps0[h][:],
                            op0=mult,
                            op1=mult,
                            accum_out=colsd[:, di : di + 1],
                        )
                        di += 1
                elif g in STT_SET:
                    nc.vector.scalar_tensor_tensor(
                        out=junkd[:],
                        in0=ds[g][:],
                        scalar=1.0,
                        in1=abqs[b][:],
                        op0=mult,
                        op1=mult,
                        accum_out=colsd[:, di : di + 1],
                    )
                    di += 1
                else:
                    prod = prodp.tile([128, S], f16)
                    if g == G_ORDER[0]:
                        # first slab: multiply per 2048-half so half 1 can
                        # start as soon as the first A copy lands (the whole
                        # DVE chain's start gate); one accum over both.
                        for h in range(2):
                            sl = slice(h * 2048, (h + 1) * 2048)
                            nc.vector.tensor_tensor(
                                out=prod[:, sl], in0=ds[g][:, sl],
                                in1=abqs[b][:, sl], op=mult)
                    else:
                        nc.vector.tensor_tensor(
                            out=prod[:], in0=ds[g][:], in1=abqs[b][:],
                            op=mult)
                    nc.scalar.activation(
                        junka[:], prod[:],
                        mybir.ActivationFunctionType.Copy,
                        accum_out=colsa[:, ai : ai + 1])
                    ai += 1

            nc.scalar.dma_start(yat[:, :], colsa[:])
            nc.sync.dma_start(ybt[:, :], colsd[:])

    nc.compile()
    return nc


def _weight_field(offset, trans_std):
    """A[b,s]: accumulated bilinear weights per source pixel (unscaled; the
    1/S mean is applied on host after download).  Mirrors the reference
    coordinate math in float32.
    """
    offset = np.asarray(offset, np.float32)
    ts = np.float32(min(max(float(trans_std), 0.001), 0.01))
    ii = np.arange(H, dtype=np.float32)[None, :, None]
    jj = np.arange(W, dtype=np.float32)[None, None, :]
    y = np.clip(ii + ts * offset[:, 0] * np.float32(H),
                np.float32(0.0), np.float32(H - 1))
    x = np.clip(jj + ts * offset[:, 1] * np.float32(W),
                np.float32(0.0), np.float32(W - 1))
    y0 = np.clip(np.floor(y).astype(np.int32), 0, H - 2)
    x0 = np.clip(np.floor(x).astype(np.int32), 0, W - 2)
    wy = (y - y0.astype(np.float32)).astype(np.float64)
    wx = (x - x0.astype(np.float32)).astype(np.float64)

    base = np.arange(offset.shape[0], dtype=np.int64)[:, None, None] * S
    i00 = (y0.astype(np.int64) * W + x0 + base).ravel()
    i01 = i00 + 1
    i10 = i00 + W
    i11 = i10 + 1
    n = offset.shape[0] * S
    acc = (
        np.bincount(i00, ((1 - wy) * (1 - wx)).ravel(), minlength=n)
        + np.bincount(i01, ((1 - wy) * wx).ravel(), minlength=n)
        + np.bincount(i10, (wy * (1 - wx)).ravel(), minlength=n)
        + np.bincount(i11, (wy * wx).ravel(), minlength=n)
    )
    return acc.astype(np.float32).reshape(offset.shape[0], S)


def _trace_available():
    """Trace only when the axon NTFF hook is registered (dev loop); in a
    bare harness environment this returns False and the run is untraced."""
    try:
        from antenv.axon_hooks import get_axon_ntff_profile_hook
    except ImportError:
        return False
    return get_axon_ntff_profile_hook() is not None


def _assemble_core(ya, yb):
    """[128, NBC] column sums from the two output tiles of one core."""
    ycols = np.empty((128, NBC), np.float32)
    ycols[:, list(TT_ORDER)] = ya
    ycols[:, STT_ORDER[0]] = yb[:, 0]
    ycols[:, STT_ORDER[1]] = yb[:, 1]
    ycols[:, PSUM_SET[0]] = yb[:, 2] + yb[:, 3]
    ycols[:, PSUM_SET[1]] = yb[:, 4] + yb[:, 5]
    return ycols


def kernel(data, offset, trans_std):
    global LAST_RESULTS
    data = np.asarray(data, np.float32)
    offset = np.asarray(offset, np.float32)
    ts = float(np.asarray(trans_std).reshape(()))

    aw16 = np.ascontiguousarray(
        _weight_field(offset, ts).astype(np.float16))    # [B, S] unscaled
    ones_pack = np.ones(128, np.float16).view(np.float32)      # [64]
    awp = np.concatenate(
        [np.tile(ones_pack, (B, 1)), aw16.view(np.float32)], axis=1)
    x16 = data.reshape(B, NCH, 128, S).astype(np.float16)

    if "nc" not in _CACHE:
        _CACHE["nc"] = _build_nc()
    nc = _CACHE["nc"]

    in_maps = []
    for i in range(N_CORES):
        xi = np.ascontiguousarray(
            x16[i * NB : (i + 1) * NB].reshape(NBC, 128, S))
        ai = np.ascontiguousarray(awp[i * NB : (i + 1) * NB])
        in_maps.append({"x": xi, "awp": ai})

    res = run_bass_kernel_spmd(nc, in_maps, core_ids=list(range(N_CORES)),
                               trace=_trace_available())
    LAST_RESULTS = res
    # y[p, g] with g = b*NCH + ch → out[b, ch*128 + p], divided by S
    parts = []
    for i in range(N_CORES):
        ycols = _assemble_core(res.results[i]["ya"], res.results[i]["yb"])
        yi = ycols.reshape(128, NB, NCH)
        parts.append(np.transpose(yi, (1, 2, 0)).reshape(NB, C))
    out = np.concatenate(parts, axis=0) * np.float32(1.0 / S)
    return np.ascontiguousarray(out.astype(np.float32))


# revision 43
# speedup vs baseline: 1.2654x; 1.0139x over previous
"""Bilinear-sampling + global average pooling on 8 Trainium2 NeuronCores.

Math: out[b,c] = mean_{h,w} bilinear(data[b,c], grid + ts*offset[b])
The gather indices/weights depend only on (b,h,w), never on c, so the whole
op is a linear functional over spatial positions applied per channel:

    out[b,c] = (1/S) * sum_s A[b,s] * data[b,c,s]      (s = flattened H*W)

where A is the scatter-accumulation of the four bilinear corner weights of
every sample point.  A is computed on host from `offset` (131K elements,
0.1% of `data`); the device kernel does the memory-bound weighted reduction
over the `data` tensor.  The 1/S mean and the final [B,C] assembly happen
on host.

The op is memory-bound, so `data` ships to HBM as fp16 (host cast): halves
DMA traffic vs fp32 at ~4e-4 relative error — far inside tolerance.

Device kernel (per core, 4 batches = 8 slabs of [128 channels, 4096]):
  - One 1 MiB HWDGE DMA per slab, deep-prefetched (all 8 issued up front on
    the Sync ring; A-row loads + result stores split across both rings).
  - A[b] rows are replicated across partitions by the TENSOR engine
    (ones[1,128] ⊗ row, 512-wide fp16 matmuls into PSUM).  Each A-row DMA
    carries a 128-one fp16 prefix so the stationary operand needs no
    separate load.  Batches 1-3 are cast-copied PSUM→SBUF fp16 by ACT;
    batch 0's PSUM tiles are consumed in place.  GPSIMD stays idle: it
    shares an exclusively-locked SBUF port with DVE, so any gpsimd op
    stretches concurrent DVE work ~2.3x.
  - Per slab, the weighted reduction runs as one of (balancing DVE vs ACT):
      DVE tensor_tensor multiply (2x perf mode) + ACT Copy-activation with
      accumulator (4 slabs),
      DVE fused scalar_tensor_tensor with accum_out (2 slabs), or
      per-2048-half fused STT reading the PSUM broadcast directly (batch 0;
      the half-sums are added on host).
  - Two small DMAs (one per ring) store the fp32 column sums.
"""

import os
import sys

import numpy as np

for _p in ("/opt/trn_rl_repo", "/root/.axon_site/_ro/trn_rl_repo"):
    if os.path.isdir(_p) and _p not in sys.path:
        sys.path.append(_p)

import concourse.bacc as bacc
import concourse.mybir as mybir
import concourse.tile as tile
from concourse.bass_utils import run_bass_kernel_spmd

N_CORES = 8
B, C, H, W = 32, 256, 64, 64
S = H * W            # 4096 spatial positions
NB = B // N_CORES    # 4 batches per core
NCH = C // 128       # 2 channel halves of 128 partitions
NBC = NB * NCH       # 8 slabs of [128, S] per core
MM_N = 512                 # moving-operand width per matmul (fp16 limit)
# All four A-row broadcasts ride PE (ones ⊗ row into PSUM).  Batches 1-3 are
# copied to SBUF fp16 (ACT) and processed first; batch 0's two slabs instead
# run fused STT reading the PSUM broadcast directly (no copies) and go last.
# Slab roles: TT+ACT reduce for TT_ORDER, fused DVE STT for STT_ORDER
# (SBUF A tile), and per-half PSUM-direct STT for slabs 0 and 1.
G_ORDER = (2, 3, 4, 5, 6, 7, 0, 1)
TT_ORDER = (2, 4, 5, 6, 7)  # → ya columns, in this order
STT_ORDER = (3,)           # → yb column 0; yb cols 1-4 are the four
                           #   half-sums of slabs 0 and 1 (host adds pairs)
STT_SET = (3,)
PSUM_SET = (0, 1)
NYB = len(STT_ORDER) + 2 * len(PSUM_SET)

_CACHE = {}
LAST_RESULTS = None  # BassKernelResults of the most recent run (for test.py)


def _build_nc():
    nc = bacc.Bacc("TRN2", target_bir_lowering=False, debug=False,
                   num_devices=N_CORES)
    f16 = mybir.dt.float16
    f32 = mybir.dt.float32
    x = nc.dram_tensor("x", [NBC, 128, S], f16, kind="ExternalInput")
    # each row: 64 fp32 = 128 fp16 ones, then the fp16 A row packed as fp32
    awp = nc.dram_tensor("awp", [NB, 64 + S // 2], f32, kind="ExternalInput")
    ya = nc.dram_tensor("ya", [128, len(TT_ORDER)], f32, kind="ExternalOutput")
    yb = nc.dram_tensor("yb", [128, NYB], f32, kind="ExternalOutput")
    xt, at, yat, ybt = x.ap(), awp.ap(), ya.ap(), yb.ap()
    mult = mybir.AluOpType.mult

    with tile.TileContext(nc) as tc:
        with (
            tc.tile_pool(name="arow", bufs=NB) as arowp,
            tc.tile_pool(name="abc", bufs=1) as abcp,
            tc.tile_pool(name="data", bufs=NBC) as datap,
            tc.tile_pool(name="prod", bufs=5) as prodp,
            tc.tile_pool(name="junk", bufs=2) as junkp,
            tc.tile_pool(name="col", bufs=2) as colp,
            tc.tile_pool(name="ps", bufs=2, space="PSUM") as psp,
        ):
            junka = junkp.tile([128, S], f16)   # ACT scratch
            junkd = junkp.tile([128, S], f16)   # DVE-STT scratch
            colsa = colp.tile([128, len(TT_ORDER)], f32)
            colsd = colp.tile([128, NYB], f32, name="colsd")

            # A-row loads (ones prefix + fp16 pairs packed as fp32): batch 1
            # first on the Sync ring — it gates the whole pipeline — then
            # batches 2,3 on Scalar and batch 0 last.
            a_sbs = [None] * NB
            with tc.high_priority():
                for b, eng in ((1, nc.sync), (2, nc.scalar),
                               (3, nc.scalar), (0, nc.sync)):
                    a_sb = arowp.tile([1, 64 + S // 2], f32,
                                      name=f"asb{b}", bufs=1)
                    eng.dma_start(a_sb[:], at[b : b + 1, :])
                    a_sbs[b] = a_sb
            onest = a_sbs[1][:, 0:64].bitcast(f16)      # [1, 128] ones

            # Broadcast A rows across partitions on PE (ones ⊗ row → PSUM).
            # Batches 1-3 are copied to SBUF fp16 by ACT; batch 0's PSUM
            # tiles are consumed directly by its fused STT slabs (GpSimd
            # stays idle — it would lock DVE out of a shared SBUF port).
            abqs = [None] * NB
            ps0 = []
            for b in (1, 2, 3, 0):
                rhs16 = a_sbs[b][:, 64:].bitcast(f16)   # [1, S] fp16 view
                if b == 0:
                    for k in range(S // 2048):
                        ps = psp.tile([128, 2048], f32, name="ps")
                        for h in range(2048 // MM_N):
                            c0 = k * 2048 + h * MM_N
                            nc.tensor.matmul(
                                ps[:, h * MM_N : (h + 1) * MM_N],
                                lhsT=onest[0:1, :],
                                rhs=rhs16[0:1, c0 : c0 + MM_N],
                                start=True, stop=True)
                        ps0.append(ps)
                    continue
                abp = abcp.tile([128, S], f16, name=f"abp{b}", bufs=1)
                for k in range(S // 2048):
                    ps = psp.tile([128, 2048], f32, name="ps")
                    for h in range(2048 // MM_N):
                        c0 = k * 2048 + h * MM_N
                        nc.tensor.matmul(
                            ps[:, h * MM_N : (h + 1) * MM_N],
                            lhsT=onest[0:1, :],
                            rhs=rhs16[0:1, c0 : c0 + MM_N],
                            start=True, stop=True)
                    nc.scalar.activation(
                        abp[:, k * 2048 : (k + 1) * 2048], ps[:],
                        mybir.ActivationFunctionType.Copy)
                abqs[b] = abp

            # Process PE-broadcast batches first: ab1 is ready ~4us before
            # the gpsimd-broadcast ab0, so batch 0 goes last.  Data DMAs are
            # issued in the same order so the stream matches consumption.
            ds = {}
            for g in G_ORDER:
                d = datap.tile([128, S], f16, name=f"d{g}", bufs=1)
                nc.sync.dma_start(d[:], xt[g])
                ds[g] = d

            ai = di = 0
            for g in G_ORDER:
                b = g // NCH
                if g in PSUM_SET:
                    # batch-0 slab: fused STT per 2048-half against the PSUM
                    # broadcast tiles; the two half-sums are added on host.
                    for h in range(2):
                        nc.vector.scalar_tensor_tensor(
                            out=junkd[:, 0:2048],
                            in0=ds[g][:, h * 2048 : (h + 1) * 2048],
                            scalar=1.0,
                            in1=ps0[h][:],
                            op0=mult,
                            op1=mult,
                            accum_out=colsd[:, di : di + 1],
                        )
                        di += 1
                elif g in STT_SET:
                    nc.vector.scalar_tensor_tensor(
                        out=junkd[:],
                        in0=ds[g][:],
                        scalar=1.0,
                        in1=abqs[b][:],
                        op0=mult,
                        op1=mult,
                        accum_out=colsd[:, di : di + 1],
                    )
                    di += 1
                else:
                    prod = prodp.tile([128, S], f16)
                    if g == G_ORDER[0]:
                        # first slab: multiply per 2048-half so half 1 can
                        # start as soon as the first A copy lands (the whole
                        # DVE chain's start gate); one accum over both.
                        for h in range(2):
                            sl = slice(h * 2048, (h + 1) * 2048)
                            nc.vector.tensor_tensor(
                                out=prod[:, sl], in0=ds[g][:, sl],
                                in1=abqs[b][:, sl], op=mult)
                    else:
                        nc.vector.tensor_tensor(
                            out=prod[:], in0=ds[g][:], in1=abqs[b][:],
                            op=mult)
                    nc.scalar.activation(
                        junka[:], prod[:],
                        mybir.ActivationFunctionType.Copy,
                        accum_out=colsa[:, ai : ai + 1])
                    ai += 1

            nc.scalar.dma_start(yat[:, :], colsa[:])
            nc.sync.dma_start(ybt[:, :], colsd[:])

    nc.compile()
    return nc


def _weight_field(offset, trans_std):
    """A[b,s]: accumulated bilinear weights per source pixel (unscaled; the
    1/S mean is applied on host after download).  Mirrors the reference
    coordinate math in float32.
    """
    offset = np.asarray(offset, np.float32)
    ts = np.float32(min(max(float(trans_std), 0.001), 0.01))
    ii = np.arange(H, dtype=np.float32)[None, :, None]
    jj = np.arange(W, dtype=np.float32)[None, None, :]
    y = np.clip(ii + ts * offset[:, 0] * np.float32(H),
                np.float32(0.0), np.float32(H - 1))
    x = np.clip(jj + ts * offset[:, 1] * np.float32(W),
                np.float32(0.0), np.float32(W - 1))
    y0 = np.clip(np.floor(y).astype(np.int32), 0, H - 2)
    x0 = np.clip(np.floor(x).astype(np.int32), 0, W - 2)
    wy = (y - y0.astype(np.float32)).astype(np.float64)
    wx = (x - x0.astype(np.float32)).astype(np.float64)

    base = np.arange(offset.shape[0], dtype=np.int64)[:, None, None] * S
    i00 = (y0.astype(np.int64) * W + x0 + base).ravel()
    i01 = i00 + 1
    i10 = i00 + W
    i11 = i10 + 1
    n = offset.shape[0] * S
    acc = (
        np.bincount(i00, ((1 - wy) * (1 - wx)).ravel(), minlength=n)
        + np.bincount(i01, ((1 - wy) * wx).ravel(), minlength=n)
        + np.bincount(i10, (wy * (1 - wx)).ravel(), minlength=n)
        + np.bincount(i11, (wy * wx).ravel(), minlength=n)
    )
    return acc.astype(np.float32).reshape(offset.shape[0], S)


def _trace_available():
    """Trace only when the axon NTFF hook is registered (dev loop); in a
    bare harness environment this returns False and the run is untraced."""
    try:
        from antenv.axon_hooks import get_axon_ntff_profile_hook
    except ImportError:
        return False
    return get_axon_ntff_profile_hook() is not None


def _assemble_core(ya, yb):
    """[128, NBC] column sums from the two output tiles of one core."""
    ycols = np.empty((128, NBC), np.float32)
    ycols[:, list(TT_ORDER)] = ya
    ycols[:, STT_ORDER[0]] = yb[:, 0]
    ycols[:, PSUM_SET[0]] = yb[:, 1] + yb[:, 2]
    ycols[:, PSUM_SET[1]] = yb[:, 3] + yb[:, 4]
    return ycols


def kernel(data, offset, trans_std):
    global LAST_RESULTS
    data = np.asarray(data, np.float32)
    offset = np.asarray(offset, np.float32)
    ts = float(np.asarray(trans_std).reshape(()))

    aw16 = np.ascontiguousarray(
        _weight_field(offset, ts).astype(np.float16))    # [B, S] unscaled
    ones_pack = np.ones(128, np.float16).view(np.float32)      # [64]
    awp = np.concatenate(
        [np.tile(ones_pack, (B, 1)), aw16.view(np.float32)], axis=1)
    x16 = data.reshape(B, NCH, 128, S).astype(np.float16)

    if "nc" not in _CACHE:
        _CACHE["nc"] = _build_nc()
    nc = _CACHE["nc"]

    in_maps = []
    for i in range(N_CORES):
        xi = np.ascontiguousarray(
            x16[i * NB : (i + 1) * NB].reshape(NBC, 128, S))
        ai = np.ascontiguousarray(awp[i * NB : (i + 1) * NB])
        in_maps.append({"x": xi, "awp": ai})

    res = run_bass_kernel_spmd(nc, in_maps, core_ids=list(range(N_CORES)),
                               trace=_trace_available())
    LAST_RESULTS = res
    # y[p, g] with g = b*NCH + ch → out[b, ch*128 + p], divided by S
    parts = []
    for i in range(N_CORES):
        ycols = _assemble_core(res.results[i]["ya"], res.results[i]["yb"])
        yi = ycols.reshape(128, NB, NCH)
        parts.append(np.transpose(yi, (1, 2, 0)).reshape(NB, C))
    out = np.concatenate(parts, axis=0) * np.float32(1.0 / S)
    return np.ascontiguousarray(out.astype(np.float32))
